# revision 13
# baseline (speedup 1.0000x reference)
"""Causal self-attention (B=1, L=4096, C=1024, H=16, D=64) on 8 TRN2 NeuronCores.

Sharding: head tensor-parallelism — each core owns 2 of the 16 heads.
Host passes per-core bf16 inputs: x transposed [C, L], the Wq/Wk/Wv column
slices and Wo row slice for the core's heads. Each core computes its partial
of out.T = Wo_local.T @ attn_local.T; the host sums the 8 partials.

v2 (p-state aware): the TRN2 PE ramps 0.65->1.2->2.4 GHz with ~3us of
continuous busy, so the kernel is organized to keep the PE queue dense:
  - warmup matmuls ramp the clock while the x DMAs stream in
  - V is produced directly in [keys, dims] layout (stationary = x chunk),
    eliminating the PE transposes of v1
  - per (j, i): both heads' S^T blocks land in one [128, 2, 512] PSUM tile;
    ONE Exp activation covers both heads (144 ACTs instead of 288)
  - PE program order is software-pipelined: S_{i+1} is issued before PV_i
    so the PE never waits on the scalar-engine exp
  - softmax normalize runs entirely off-PE: denominator row -> SBUF,
    gpsimd partition_broadcast, wide [64, 512] reciprocal, one multiply
  - attn output accumulates in SBUF (att2 [128, L]); Wo matmuls + output
    casts (round-robin vector/scalar/gpsimd) + 1MB-batched DMAs form a
    final phase that reuses the score-PSUM slots
"""
import math
import sys
from contextlib import ExitStack

import numpy as np

sys.path.insert(0, "/opt/trn_rl_repo")

import ml_dtypes  # noqa: E402

import concourse.bass as bass  # noqa: E402,F401
import concourse.mybir as mybir  # noqa: E402
import concourse.tile as tile  # noqa: E402
from concourse import bacc  # noqa: E402

FP32 = mybir.dt.float32
BF16 = mybir.dt.bfloat16
MASK_VAL = -30000.0

L, C, H, D = 4096, 1024, 16, 64
N_CORES = 8


def _build_nc():
    DH2, QB, KB = 128, 512, 128
    NQ = L // QB
    NCC = C // 128
    SUB = QB // KB
    NKB = L // KB
    scale = 1.0 / math.sqrt(D)
    Exp = mybir.ActivationFunctionType.Exp
    Copy = mybir.ActivationFunctionType.Copy

    nc = bacc.Bacc("TRN2", target_bir_lowering=False, debug=False,
                   num_devices=N_CORES)
    xT = nc.declare_dram_parameter("xT", [C, L], BF16, isOutput=False)
    # [128, NCC*DH2] chunk-major (host pre-reshaped) -> one contiguous DMA
    wq = nc.declare_dram_parameter("wq", [128, C], BF16, isOutput=False)
    wk = nc.declare_dram_parameter("wk", [128, C], BF16, isOutput=False)
    wv = nc.declare_dram_parameter("wv", [128, C], BF16, isOutput=False)
    wo = nc.declare_dram_parameter("wo", [DH2, C], BF16, isOutput=False)
    outT = nc.declare_dram_parameter("outT", [C, L], BF16, isOutput=True)

    with tile.TileContext(nc) as tc, ExitStack() as ctx:
        big = ctx.enter_context(tc.tile_pool(name="big", bufs=1))
        work = ctx.enter_context(tc.tile_pool(name="work", bufs=4))

        warmf = big.tile([128, 512], FP32, tag="warmf")
        nc.gpsimd.memset(warmf[:], 0.0)

        xt_sb = big.tile([128, NCC, L], BF16, tag="xt")
        for c in range(NCC):
            nc.sync.dma_start(xt_sb[:, c, 0:L // 2],
                              xT[c * 128:(c + 1) * 128, 0:L // 2])
            nc.sync.dma_start(xt_sb[:, c, L // 2:L],
                              xT[c * 128:(c + 1) * 128, L // 2:L])
        wq_sb = big.tile([128, NCC, DH2], BF16, tag="wq")
        wk_sb = big.tile([128, NCC, DH2], BF16, tag="wk")
        wv_sb = big.tile([128, NCC, DH2], BF16, tag="wv")
        for w_sb, w_dram in ((wq_sb, wq), (wk_sb, wk), (wv_sb, wv)):
            nc.sync.dma_start(
                w_sb[:], w_dram.rearrange("p (n d) -> p n d", n=NCC))
        wo_sb = big.tile([128, C], BF16, tag="wo")
        nc.sync.dma_start(wo_sb[:], wo[:])

        with tc.tile_pool(name="psQ", bufs=2, space="PSUM") as psQ:
            # Dummy fp32 matmuls (~4x slower per col) ramp the PE p-state
            # toward 2.4GHz while the x/weight DMAs stream in (~35us).
            for _ in range(22):
                wp = psQ.tile([128, 512], FP32, tag="pp")
                nc.tensor.matmul(wp[:], warmf[:, 0:128], warmf[:],
                                 start=True, stop=True)

            qt2 = big.tile([128, L], BF16, tag="qt2")
            kt2 = big.tile([128, L], BF16, tag="kt2")
            for dst, w_sb in ((qt2, wq_sb), (kt2, wk_sb)):
                for b in range(L // 512):
                    pp = psQ.tile([128, 512], FP32, tag="pp")
                    for c in range(NCC):
                        nc.tensor.matmul(pp[:], w_sb[:, c, :],
                                         xt_sb[:, c, b * 512:(b + 1) * 512],
                                         start=(c == 0), stop=(c == NCC - 1))
                    nc.vector.tensor_copy(dst[:, b * 512:(b + 1) * 512], pp[:])

            # V directly in [keys, dims] layout: stationary = x chunk block,
            # moving = Wv chunk. vaug col layout per 128-key block:
            # 0-63 V_h0 | 64 ones | 65-128 V_h1 | 129 ones.
            vaug = big.tile([128, NKB, 130], BF16, tag="vaug")
            nc.gpsimd.memset(vaug[:, :, 64:65], 1.0)
            nc.gpsimd.memset(vaug[:, :, 129:130], 1.0)
            for vb in range(L // 512):
                vp = psQ.tile([128, 512], FP32, tag="pp")
                for s in range(4):
                    l0 = vb * 512 + s * 128
                    for c in range(NCC):
                        nc.tensor.matmul(vp[:, s * 128:(s + 1) * 128],
                                         xt_sb[:, c, l0:l0 + 128],
                                         wv_sb[:, c, :],
                                         start=(c == 0), stop=(c == NCC - 1))
                for s in range(4):
                    i = vb * 4 + s
                    nc.vector.tensor_copy(vaug[:, i, 0:64],
                                          vp[:, s * 128:s * 128 + 64])
                    nc.vector.tensor_copy(vaug[:, i, 65:129],
                                          vp[:, s * 128 + 64:(s + 1) * 128])

        att2 = big.tile([128, L], BF16, tag="att2")
        psB = ctx.enter_context(tc.tile_pool(name="psB", bufs=3, space="PSUM"))
        psO = ctx.enter_context(tc.tile_pool(name="psO", bufs=1, space="PSUM"))

        # Wo work interleaved into the (exp-bound) attention loop: each job
        # is two output-chunk matmuls for a finished q-block + one wide cast;
        # at most one job per 3 iterations so the pw tile's st-slot reuse
        # stays ahead of its cast.
        outT_re = outT.rearrange("(n p) q -> p n q", p=128)
        wo_state = {"jobs": [], "ob": {}, "done": {}}

        def emit_wo_job():
            if not wo_state["jobs"]:
                return
            jj, pair = wo_state["jobs"].pop(0)
            if jj not in wo_state["ob"]:
                wo_state["ob"][jj] = work.tile([128, NCC, QB], BF16,
                                               tag="ob", bufs=2, name="ob")
                wo_state["done"][jj] = 0
            ob = wo_state["ob"][jj]
            pw = psB.tile([128, 2, QB], FP32, tag="st", name="pw")
            for t in range(2):
                cc = pair * 2 + t
                nc.tensor.matmul(pw[:, t, :],
                                 wo_sb[:, cc * 128:(cc + 1) * 128],
                                 att2[:, jj * QB:(jj + 1) * QB],
                                 start=True, stop=True)
            nc.vector.tensor_copy(ob[:, pair * 2:pair * 2 + 2, :], pw[:])
            wo_state["done"][jj] += 1
            if wo_state["done"][jj] == NCC // 2:
                nc.sync.dma_start(outT_re[:, :, jj * QB:(jj + 1) * QB], ob[:])
                del wo_state["ob"][jj]

        for j in range(NQ):
            nk = (j + 1) * SUB
            o_ps = [psO.tile([65, QB], FP32, tag=f"o{h}", name=f"o_ps{h}")
                    for h in range(2)]
            pts = {}

            def emit_S(i):
                c0 = max(0, (i - j * SUB)) * KB
                st = psB.tile([128, 2, QB], FP32, tag="st", name="st")
                for h in range(2):
                    r0, r1 = h * D, (h + 1) * D
                    nc.tensor.matmul(st[:, h, c0:QB],
                                     kt2[r0:r1, i * KB:(i + 1) * KB],
                                     qt2[r0:r1, j * QB + c0:(j + 1) * QB],
                                     start=True, stop=True)
                pt = work.tile([128, 2, QB], BF16, tag="pt", bufs=4,
                               name="pt")
                nc.scalar.activation(pt[:, :, c0:QB], st[:, :, c0:QB], Exp,
                                     scale=scale)
                if i >= j * SUB:
                    # causal mask applied post-exp: zero the upper triangle
                    # of the diagonal block on the (otherwise idle) gpsimd,
                    # keeping the exp stream free of cross-engine waits
                    for h in range(2):
                        nc.gpsimd.affine_select(
                            out=pt[:, h, c0:c0 + KB],
                            in_=pt[:, h, c0:c0 + KB],
                            compare_op=mybir.AluOpType.is_ge,
                            fill=0.0, base=0,
                            pattern=[[1, KB]], channel_multiplier=-1)
                pts[i] = (pt, c0)

            def emit_PV(i):
                pt, c0 = pts.pop(i)
                for h in range(2):
                    nc.tensor.matmul(o_ps[h][:, c0:QB],
                                     vaug[:, i, h * 65:h * 65 + 65],
                                     pt[:, h, c0:QB],
                                     start=(i == 0), stop=(i == nk - 1))

            emit_S(0)
            if nk > 1:
                emit_S(1)
            for i in range(2, nk):
                emit_S(i)
                emit_PV(i - 2)
                if i >= 3 and i % 3 == 0:
                    emit_wo_job()
            if nk > 1:
                emit_PV(nk - 2)
            emit_PV(nk - 1)

            # fast drain: cast both heads' accumulators to SBUF so the
            # single-buffered PSUM slots free before j+1's first PV
            o_sb = []
            for h in range(2):
                ot = work.tile([65, QB], FP32, tag=f"osb{h}", bufs=2)
                nc.vector.tensor_copy(ot[:], o_ps[h][:])
                o_sb.append(ot)
            # normalize off-PE: broadcast denom on gpsimd, fast reciprocal
            for h in range(2):
                r0, r1 = h * D, (h + 1) * D
                den = work.tile([1, QB], FP32, tag=f"den{h}", bufs=2)
                nc.vector.tensor_copy(den[:], o_sb[h][64:65, :])
                bc = work.tile([64, QB], FP32, tag=f"bc{h}", bufs=2)
                nc.gpsimd.partition_broadcast(bc[:], den[:], channels=64)
                rec = work.tile([64, QB], FP32, tag=f"rec{h}", bufs=2)
                nc.vector.reciprocal_approx_fast(rec[:], bc[:])
                nc.vector.tensor_mul(att2[r0:r1, j * QB:(j + 1) * QB],
                                     o_sb[h][0:64, :], rec[:])
            wo_state["jobs"].extend((j, p) for p in range(NCC // 2))

        while wo_state["jobs"]:
            emit_wo_job()
    nc.compile()
    return nc


_NC_CACHE = None


def _get_nc():
    global _NC_CACHE
    if _NC_CACHE is None:
        _NC_CACHE = _build_nc()
    return _NC_CACHE


def _chunk_major(w):
    """[1024, 128] -> [128, 8*128]: element [p, n*128+d] = w[n*128+p, d]."""
    return np.ascontiguousarray(
        w.reshape(8, 128, 128).transpose(1, 0, 2).reshape(128, 1024))


def make_in_maps(x, Wq, Wk, Wv, Wo):
    bf16 = ml_dtypes.bfloat16
    x = np.asarray(x, np.float32).reshape(L, C)
    xT = np.ascontiguousarray(x.T).astype(bf16)
    Wq, Wk, Wv, Wo = (np.asarray(w, np.float32) for w in (Wq, Wk, Wv, Wo))
    in_maps = []
    for c in range(N_CORES):
        cols = slice(128 * c, 128 * (c + 1))
        in_maps.append({
            "xT": xT,
            "wq": _chunk_major(Wq[:, cols]).astype(bf16),
            "wk": _chunk_major(Wk[:, cols]).astype(bf16),
            "wv": _chunk_major(Wv[:, cols]).astype(bf16),
            "wo": np.ascontiguousarray(Wo[cols, :]).astype(bf16),
        })
    return in_maps


def combine_results(results):
    acc = np.zeros((C, L), np.float32)
    for r in results:
        acc += np.asarray(r["outT"], np.float32)
    return np.ascontiguousarray(acc.T)[None].astype(np.float32)


def kernel(x, Wq, Wk, Wv, Wo):
    from concourse.bass_utils import run_bass_kernel_spmd
    nc = _get_nc()
    in_maps = make_in_maps(x, Wq, Wk, Wv, Wo)
    res = run_bass_kernel_spmd(nc, in_maps, core_ids=list(range(N_CORES)))
    return combine_results(res.results)


# revision 17
# speedup vs baseline: 1.0416x; 1.0416x over previous
"""Causal self-attention (B=1, L=4096, C=1024, H=16, D=64) on 8 TRN2 NeuronCores.

Sharding: head tensor-parallelism — each core owns 2 of the 16 heads.
Host passes per-core bf16 inputs: x transposed [C, L], the Wq/Wk/Wv column
slices and Wo row slice for the core's heads. Each core computes its partial
of out.T = Wo_local.T @ attn_local.T; the host sums the 8 partials.

v2 (p-state aware): the TRN2 PE ramps 0.65->1.2->2.4 GHz with ~3us of
continuous busy, so the kernel is organized to keep the PE queue dense:
  - warmup matmuls ramp the clock while the x DMAs stream in
  - V is produced directly in [keys, dims] layout (stationary = x chunk),
    eliminating the PE transposes of v1
  - per (j, i): both heads' S^T blocks land in one [128, 2, 512] PSUM tile;
    ONE Exp activation covers both heads (144 ACTs instead of 288)
  - PE program order is software-pipelined: S_{i+1} is issued before PV_i
    so the PE never waits on the scalar-engine exp
  - softmax normalize runs entirely off-PE: denominator row -> SBUF,
    gpsimd partition_broadcast, wide [64, 512] reciprocal, one multiply
  - attn output accumulates in SBUF (att2 [128, L]); Wo matmuls + output
    casts (round-robin vector/scalar/gpsimd) + 1MB-batched DMAs form a
    final phase that reuses the score-PSUM slots
"""
import math
import sys
from contextlib import ExitStack

import numpy as np

sys.path.insert(0, "/opt/trn_rl_repo")

import ml_dtypes  # noqa: E402

import concourse.bass as bass  # noqa: E402,F401
import concourse.mybir as mybir  # noqa: E402
import concourse.tile as tile  # noqa: E402
from concourse import bacc  # noqa: E402

FP32 = mybir.dt.float32
BF16 = mybir.dt.bfloat16
MASK_VAL = -30000.0

L, C, H, D = 4096, 1024, 16, 64
N_CORES = 8


def _build_nc():
    DH2, QB, KB = 128, 512, 128
    NQ = L // QB
    NCC = C // 128
    SUB = QB // KB
    NKB = L // KB
    scale = 1.0 / math.sqrt(D)
    Exp = mybir.ActivationFunctionType.Exp
    Copy = mybir.ActivationFunctionType.Copy

    nc = bacc.Bacc("TRN2", target_bir_lowering=False, debug=False,
                   num_devices=N_CORES)
    xT = nc.declare_dram_parameter("xT", [C, L], BF16, isOutput=False)
    # [128, NCC*DH2] chunk-major (host pre-reshaped) -> one contiguous DMA
    wq = nc.declare_dram_parameter("wq", [128, C], BF16, isOutput=False)
    wk = nc.declare_dram_parameter("wk", [128, C], BF16, isOutput=False)
    wv = nc.declare_dram_parameter("wv", [128, C], BF16, isOutput=False)
    wo = nc.declare_dram_parameter("wo", [DH2, C], BF16, isOutput=False)
    outT = nc.declare_dram_parameter("outT", [C, L], BF16, isOutput=True)

    with tile.TileContext(nc) as tc, ExitStack() as ctx:
        big = ctx.enter_context(tc.tile_pool(name="big", bufs=1))
        work = ctx.enter_context(tc.tile_pool(name="work", bufs=4))

        warmf = big.tile([128, 512], FP32, tag="warmf")
        nc.gpsimd.memset(warmf[:], 0.0)

        xt_sb = big.tile([128, NCC, L], BF16, tag="xt")
        for c in range(NCC):
            nc.sync.dma_start(xt_sb[:, c, 0:L // 2],
                              xT[c * 128:(c + 1) * 128, 0:L // 2])
            nc.sync.dma_start(xt_sb[:, c, L // 2:L],
                              xT[c * 128:(c + 1) * 128, L // 2:L])
        wq_sb = big.tile([128, NCC, DH2], BF16, tag="wq")
        wk_sb = big.tile([128, NCC, DH2], BF16, tag="wk")
        wv_sb = big.tile([128, NCC, DH2], BF16, tag="wv")
        for w_sb, w_dram in ((wq_sb, wq), (wk_sb, wk), (wv_sb, wv)):
            nc.sync.dma_start(
                w_sb[:], w_dram.rearrange("p (n d) -> p n d", n=NCC))
        wo_sb = big.tile([128, C], BF16, tag="wo")
        nc.sync.dma_start(wo_sb[:], wo[:])

        with tc.tile_pool(name="psQ", bufs=2, space="PSUM") as psQ:
            # Dummy fp32 matmuls (~4x slower per col) ramp the PE p-state
            # toward 2.4GHz while the x/weight DMAs stream in (~35us).
            for _ in range(22):
                wp = psQ.tile([128, 512], FP32, tag="pp")
                nc.tensor.matmul(wp[:], warmf[:, 0:128], warmf[:],
                                 start=True, stop=True)

            qt2 = big.tile([128, L], BF16, tag="qt2")
            kt2 = big.tile([128, L], BF16, tag="kt2")
            for dst, w_sb in ((qt2, wq_sb), (kt2, wk_sb)):
                for b in range(L // 512):
                    pp = psQ.tile([128, 512], FP32, tag="pp")
                    for c in range(NCC):
                        nc.tensor.matmul(pp[:], w_sb[:, c, :],
                                         xt_sb[:, c, b * 512:(b + 1) * 512],
                                         start=(c == 0), stop=(c == NCC - 1))
                    nc.vector.tensor_copy(dst[:, b * 512:(b + 1) * 512], pp[:])

            # V directly in [keys, dims] layout: stationary = x chunk block,
            # moving = Wv chunk. vaug col layout per 128-key block:
            # 0-63 V_h0 | 64 ones | 65-128 V_h1 | 129 ones.
            vaug = big.tile([128, NKB, 130], BF16, tag="vaug")
            nc.gpsimd.memset(vaug[:, :, 64:65], 1.0)
            nc.gpsimd.memset(vaug[:, :, 129:130], 1.0)
            for vb in range(L // 512):
                vp = psQ.tile([128, 512], FP32, tag="pp")
                for s in range(4):
                    l0 = vb * 512 + s * 128
                    for c in range(NCC):
                        nc.tensor.matmul(vp[:, s * 128:(s + 1) * 128],
                                         xt_sb[:, c, l0:l0 + 128],
                                         wv_sb[:, c, :],
                                         start=(c == 0), stop=(c == NCC - 1))
                for s in range(4):
                    i = vb * 4 + s
                    nc.vector.tensor_copy(vaug[:, i, 0:64],
                                          vp[:, s * 128:s * 128 + 64])
                    nc.vector.tensor_copy(vaug[:, i, 65:129],
                                          vp[:, s * 128 + 64:(s + 1) * 128])

        att2 = big.tile([128, L], BF16, tag="att2")
        loop_ps = ExitStack()
        psB = loop_ps.enter_context(
            tc.tile_pool(name="psB", bufs=3, space="PSUM"))
        psO = loop_ps.enter_context(
            tc.tile_pool(name="psO", bufs=1, space="PSUM"))

        for j in range(NQ):
            nk = (j + 1) * SUB
            o_ps = [psO.tile([65, QB], FP32, tag=f"o{h}", name=f"o_ps{h}")
                    for h in range(2)]
            pts = {}

            def emit_S(i):
                c0 = max(0, (i - j * SUB)) * KB
                st = psB.tile([128, 2, QB], FP32, tag="st", name="st")
                for h in range(2):
                    r0, r1 = h * D, (h + 1) * D
                    nc.tensor.matmul(st[:, h, c0:QB],
                                     kt2[r0:r1, i * KB:(i + 1) * KB],
                                     qt2[r0:r1, j * QB + c0:(j + 1) * QB],
                                     start=True, stop=True)
                pt = work.tile([128, 2, QB], BF16, tag="pt", bufs=4,
                               name="pt")
                nc.scalar.activation(pt[:, :, c0:QB], st[:, :, c0:QB], Exp,
                                     scale=scale)
                if i >= j * SUB:
                    # causal mask applied post-exp: zero the upper triangle
                    # of the diagonal block on the (otherwise idle) gpsimd,
                    # keeping the exp stream free of cross-engine waits
                    for h in range(2):
                        nc.gpsimd.affine_select(
                            out=pt[:, h, c0:c0 + KB],
                            in_=pt[:, h, c0:c0 + KB],
                            compare_op=mybir.AluOpType.is_ge,
                            fill=0.0, base=0,
                            pattern=[[1, KB]], channel_multiplier=-1)
                pts[i] = (pt, c0)

            def emit_PV(i):
                pt, c0 = pts.pop(i)
                for h in range(2):
                    nc.tensor.matmul(o_ps[h][:, c0:QB],
                                     vaug[:, i, h * 65:h * 65 + 65],
                                     pt[:, h, c0:QB],
                                     start=(i == 0), stop=(i == nk - 1))

            emit_S(0)
            if nk > 1:
                emit_S(1)
            for i in range(2, nk):
                emit_S(i)
                emit_PV(i - 2)
            if nk > 1:
                emit_PV(nk - 2)
            emit_PV(nk - 1)

            # fast drain: cast both heads' accumulators to SBUF so the
            # single-buffered PSUM slots free before j+1's first PV
            o_sb = []
            for h in range(2):
                ot = work.tile([65, QB], FP32, tag=f"osb{h}", bufs=2)
                nc.vector.tensor_copy(ot[:], o_ps[h][:])
                o_sb.append(ot)
            # normalize off-PE: broadcast denom on gpsimd, fast reciprocal
            for h in range(2):
                r0, r1 = h * D, (h + 1) * D
                den = work.tile([1, QB], FP32, tag=f"den{h}", bufs=2)
                nc.vector.tensor_copy(den[:], o_sb[h][64:65, :])
                bc = work.tile([64, QB], FP32, tag=f"bc{h}", bufs=2)
                nc.gpsimd.partition_broadcast(bc[:], den[:], channels=64)
                rec = work.tile([64, QB], FP32, tag=f"rec{h}", bufs=2)
                nc.vector.reciprocal_approx_fast(rec[:], bc[:])
                nc.vector.tensor_mul(att2[r0:r1, j * QB:(j + 1) * QB],
                                     o_sb[h][0:64, :], rec[:])

        # Wo phase: release the attention PSUM pools; wide casts alternate
        # vector/scalar; one 1MB DMA per 128-row output chunk.
        loop_ps.close()
        with tc.tile_pool(name="psW", bufs=3, space="PSUM") as psW:
            k = 0
            for cc in range(NCC):
                ob = work.tile([128, NQ, QB], BF16, tag="ob", bufs=3,
                               name="ob")
                for j2 in range(0, NQ, 2):
                    pw = psW.tile([128, 2, QB], FP32, tag="pw", name="pw")
                    for t in range(2):
                        jj = j2 + t
                        nc.tensor.matmul(pw[:, t, :],
                                         wo_sb[:, cc * 128:(cc + 1) * 128],
                                         att2[:, jj * QB:(jj + 1) * QB],
                                         start=True, stop=True)
                    if k % 2 == 0:
                        nc.vector.tensor_copy(ob[:, j2:j2 + 2, :], pw[:])
                    else:
                        nc.scalar.activation(ob[:, j2:j2 + 2, :], pw[:],
                                             Copy)
                    k += 1
                nc.sync.dma_start(outT[cc * 128:(cc + 1) * 128, :], ob[:])
    nc.compile()
    return nc


_NC_CACHE = None


def _get_nc():
    global _NC_CACHE
    if _NC_CACHE is None:
        _NC_CACHE = _build_nc()
    return _NC_CACHE


def _chunk_major(w):
    """[1024, 128] -> [128, 8*128]: element [p, n*128+d] = w[n*128+p, d]."""
    return np.ascontiguousarray(
        w.reshape(8, 128, 128).transpose(1, 0, 2).reshape(128, 1024))


def make_in_maps(x, Wq, Wk, Wv, Wo):
    bf16 = ml_dtypes.bfloat16
    x = np.asarray(x, np.float32).reshape(L, C)
    xT = np.ascontiguousarray(x.T).astype(bf16)
    Wq, Wk, Wv, Wo = (np.asarray(w, np.float32) for w in (Wq, Wk, Wv, Wo))
    in_maps = []
    for c in range(N_CORES):
        cols = slice(128 * c, 128 * (c + 1))
        in_maps.append({
            "xT": xT,
            "wq": _chunk_major(Wq[:, cols]).astype(bf16),
            "wk": _chunk_major(Wk[:, cols]).astype(bf16),
            "wv": _chunk_major(Wv[:, cols]).astype(bf16),
            "wo": np.ascontiguousarray(Wo[cols, :]).astype(bf16),
        })
    return in_maps


def combine_results(results):
    acc = np.zeros((C, L), np.float32)
    for r in results:
        acc += np.asarray(r["outT"], np.float32)
    return np.ascontiguousarray(acc.T)[None].astype(np.float32)


def kernel(x, Wq, Wk, Wv, Wo):
    from concourse.bass_utils import run_bass_kernel_spmd
    nc = _get_nc()
    in_maps = make_in_maps(x, Wq, Wk, Wv, Wo)
    res = run_bass_kernel_spmd(nc, in_maps, core_ids=list(range(N_CORES)))
    return combine_results(res.results)
